# revision 75
# baseline (speedup 1.0000x reference)
"""Chunked GLA forward (nn_Gen2SingleInputReadout) as a Trainium2 Bass/Tile kernel.

Math (per batch element b, per chunk of C=128 timesteps):
    v = x @ Wv^T + bv                         (T, d=512)
    k/q = x @ W^T + b                         (T, n=128)
    alpha = sigmoid(x @ Wa^T + ba)            (T, n)
    cp[t]   = cumprod(alpha) within chunk
    invp[t] = 1 / (cp[t] + EPS)
    A[t,s]  = sum_n (q[t]*cp[t])_n * (k[s]*invp[s])_n ,  masked s<=t
    y[t]    = sum_{s<=t} A[t,s] v[s]

The inter-chunk state term decays by cumprod(alpha) over >=1 full chunk
(< 1e-28) and is dropped; chunks are independent (same as the reference's
effective numerics: its 1/(cp+1e-8) saturation kills cross-chunk terms too).

Precision scheme (validated against an fp64 oracle, gate = max|dy|/max|y|):
  - All x@W projections run as fp8 DoubleRow matmuls (2 k-tiles of 128
    contracted per instruction at 0.5 cyc/row).
  - v/k/q need ~bf16 accuracy -> 3-term hi/lo compensation:
        x @ W ~= xh@Wh + xl@Wh + xh@Wl
    with xh/Wh in e4m3 and the residuals xl/Wl in e5m2 (wide exponent range
    covers the small residuals; all 3 terms accumulate in one PSUM group).
    Measured gate error ~1e-3.
  - The gate projection za tolerates pure fp8 (errors enter through decay
    RATIOS cp[t]/cp[s] which cancel): xh@Wh only. Measured ~6e-3 combined.
  - Weights are pre-scaled x64 on the host (e4m3 min-normal is 2^-6; raw
    W~0.02 would quantize terribly). The x64 cancels: za via ACT sigmoid
    scale=1/64; k~/q~ via x64-scaled biases; V's x64 via the attention mask
    value 2^-18 = 1/(64*64^2) (k~*q~ carry 64^2, V carries 64).
  - Attention stays f32r; y is written out as fp16 (halves output DMA).

Sharding: batch B=8 -> one batch element per NeuronCore (8 cores).

Engine budget per chunk-pair (256 t), steady state @2.4GHz:
    PE   6400 cyc = 2667ns  (za 256, k/q 768 ea, V 3072, AT 512, y 1024)
    DVE  ~2400ns  (atm x2, scan x2, +eps, recip, stt k~, stt q~)
    ACT  ~1500ns  (sigmoid, y evac x2 -> fp16)
    Pool ~1550ns  (V evac x2, PSUM->SBUF copies)
    DMA  ~1250ns  (x-hi/lo blocks amortized, y out)
"""

import numpy as np

import concourse.bass as bass
import concourse.bacc as bacc
import concourse.tile as tile
import concourse.mybir as mybir
from concourse.bass_utils import run_bass_kernel_spmd
from concourse.masks import make_upper_triangular

F32 = mybir.dt.float32
F32R = mybir.dt.float32r
F16 = mybir.dt.float16
FP8H = mybir.dt.float8e4   # e4m3: hi parts
FP8L = mybir.dt.float8e5   # e5m2: residuals (wide exponent range)
AF = mybir.ActivationFunctionType
ALU = mybir.AluOpType
DR = mybir.MatmulPerfMode.DoubleRow

T, B, I = 2048, 8, 512      # time, batch, in_dim
D, N = 512, 128             # d_value, d_key
C = 128                     # chunk
NCH = T // C                # 16 chunks
NPAIR = NCH // 2            # 8 chunk pairs
EPS = 1e-8
NCORES = 8

WS = 64.0                   # host-side weight pre-scale
UVAL = 1.0 / (WS ** 3)      # mask value folds away all the x64 scales
N_WARM = 12                 # PE warm-up matmuls (ramp + cover input DMA wait)

# weight-pack column layout: [Wv 0:512 | Wk 512:640 | Wq 640:768]; Wa separate
CV, CK, CQ = slice(0, 512), slice(512, 640), slice(640, 768)


def build_nc(bv_zero=True):
    nc = bacc.Bacc("TRN2", target_bir_lowering=False, debug=False)

    xh = nc.dram_tensor("xh", [I, T], FP8H, kind="ExternalInput")
    xl = nc.dram_tensor("xl", [I, T], FP8L, kind="ExternalInput")
    wahi = nc.dram_tensor("wahi", [I, N], FP8H, kind="ExternalInput")
    whi = nc.dram_tensor("whi", [I, 768], FP8H, kind="ExternalInput")
    wlo = nc.dram_tensor("wlo", [I, 768], FP8L, kind="ExternalInput")
    bias = nc.dram_tensor("bias", [N, 3], F32, kind="ExternalInput")  # ba|bk64|bq64
    bv64 = nc.dram_tensor("bv64", [1, D], F32, kind="ExternalInput")
    y = nc.dram_tensor("y", [T, D], F16, kind="ExternalOutput")

    with tile.TileContext(nc) as tc:
        _emit(tc, xh, xl, wahi, whi, wlo, bias, bv64, y, bv_zero)
    nc.compile()
    return nc


def _emit(tc, xh, xl, wahi, whi, wlo, bias, bv64, y, bv_zero):
    nc = tc.nc
    import contextlib

    ctx = contextlib.ExitStack()
    const = ctx.enter_context(tc.tile_pool(name="const", bufs=1))
    work = ctx.enter_context(tc.tile_pool(name="work", bufs=5))
    gate = ctx.enter_context(tc.tile_pool(name="gate", bufs=6))
    vout = ctx.enter_context(tc.tile_pool(name="vout", bufs=6))
    yout = ctx.enter_context(tc.tile_pool(name="yout", bufs=4))
    ps_za = ctx.enter_context(tc.tile_pool(name="ps_za", bufs=1, space="PSUM"))
    ps_kq = ctx.enter_context(tc.tile_pool(name="ps_kq", bufs=2, space="PSUM"))
    ps_v = ctx.enter_context(tc.tile_pool(name="ps_v", bufs=2, space="PSUM"))
    ps_at = ctx.enter_context(tc.tile_pool(name="ps_at", bufs=1, space="PSUM"))
    ps_y = ctx.enter_context(tc.tile_pool(name="ps_y", bufs=2, space="PSUM"))

    with ctx:
        # ---- inputs ----------------------------------------------------
        # SP HWDGE queue, ordered so pair 0's inputs land first and in the
        # order its matmul terms consume them (hh needs wahi/whi + xh0,
        # hl needs wlo, lh needs xl0).  512-col x blocks keep descriptors
        # at 512B (no sub-512B DMA penalty).
        bias_sb = const.tile([N, 3], F32, tag="bias", name="bias")
        nc.sync.dma_start(bias_sb[:], bias[:])

        xh_sb = const.tile([128, 4, T], FP8H, tag="xh", name="xh")
        xl_sb = const.tile([128, 4, T], FP8L, tag="xl", name="xl")

        def x_block(dst, src, blk):
            cs = slice(blk * 512, (blk + 1) * 512)
            nc.sync.dma_start(
                dst[:, :, cs], src[:, cs].rearrange("(j p) t -> p j t", p=128)
            )

        x_block(xh_sb, xh, 0)
        wahi_sb = const.tile([128, 4, N], FP8H, tag="wahi", name="wahi")
        nc.sync.dma_start(wahi_sb[:], wahi.rearrange("(j p) n -> p j n", p=128))
        whi_sb = const.tile([128, 4, 768], FP8H, tag="whi", name="whi")
        nc.sync.dma_start(whi_sb[:], whi.rearrange("(j p) n -> p j n", p=128))
        wlo_sb = const.tile([128, 4, 768], FP8L, tag="wlo", name="wlo")
        nc.sync.dma_start(wlo_sb[:], wlo.rearrange("(j p) n -> p j n", p=128))
        x_block(xl_sb, xl, 0)
        for blk in range(1, 4):
            x_block(xh_sb, xh, blk)
            x_block(xl_sb, xl, blk)

        bv_full = None
        if not bv_zero:
            bv_sb = const.tile([1, D], F32, tag="bv", name="bv")
            nc.scalar.dma_start(bv_sb[:], bv64[:])
            bv_full = const.tile([C, D], F32, tag="bvfull", name="bvfull")
            nc.gpsimd.partition_broadcast(bv_full[:], bv_sb[:])

        warm_z = const.tile([128, 256], mybir.dt.bfloat16, tag="warmz", name="warmz")
        nc.gpsimd.memset(warm_z[:], 0.0)
        U = const.tile([C, C], F32, tag="umask", name="umask")  # U[s,t]=UVAL iff s<=t
        make_upper_triangular(nc, U[:], val=UVAL, diag=True)
        zeros = const.tile([128, C], F32, tag="zeros", name="zeros")
        nc.vector.memset(zeros[:], 0.0)

        # PE warm-up on the zeros tile: keeps PE continuously busy from t~0
        # so the p-state ramp completes during the input DMA wait.
        warm = ps_y.tile([C, D], F32, tag="y", name="warm")
        for _ in range(N_WARM):
            nc.tensor.matmul(warm[:, 0:256], warm_z[:, 0:128], warm_z[:],
                             start=True, stop=True)

        st = {
            "tc": tc,
            "xh": xh_sb, "xl": xl_sb, "wahi": wahi_sb, "whi": whi_sb,
            "wlo": wlo_sb,
            "ba": bias_sb[:, 0:1], "bk64": bias_sb[:, 1:2], "bq64": bias_sb[:, 2:3],
            "bv_full": bv_full, "U": U, "zeros": zeros,
            "work": work, "gate": gate, "vout": vout, "yout": yout,
            "ps_za": ps_za, "ps_kq": ps_kq, "ps_v": ps_v,
            "ps_at": ps_at, "ps_y": ps_y, "y": y,
        }

        # ---- software-pipelined pair loop (stage C two pairs behind, so
        # the ~2.4us gate-chain latency of pair p is fully hidden).  The
        # last stage_a iteration emits in halves with stage_c of p-1
        # interleaved, so the in-order engines drain the pipeline instead
        # of serializing c(NPAIR-2), c(NPAIR-1) behind all of a(NPAIR-1).
        DELAY = 2
        pending = []
        for it in range(NPAIR + DELAY):
            if it >= DELAY:
                _emit_stage_c(nc, pending[it - DELAY], st)
            if it < NPAIR:
                pending.append(_emit_stage_a(nc, it, st))


def _hilo_proj(nc, out, lhs_of, rhs_of, pure=False):
    """Emit the fp8 DoubleRow matmul group for one projection.

    lhs_of/rhs_of: callables (part, u) -> AP giving the [128, 2, *] k-tile
    pair slice for term part in {'hh','lh','hl'} and K-block pair u in {0,1}.
    pure=True emits only the hi@hi term.
    """
    parts = ["hh"] if pure else ["hh", "hl", "lh"]
    seq = [(part, u) for part in parts for u in range(2)]
    for i, (part, u) in enumerate(seq):
        nc.tensor.matmul(out, lhs_of(part, u), rhs_of(part, u),
                         start=(i == 0), stop=(i == len(seq) - 1), perf_mode=DR)


def _emit_stage_a(nc, p, st, mid=None):
    """Projections + gate chain for chunk pair p."""
    work, gate, vout = st["work"], st["gate"], st["vout"]
    xh, xl, whi, wlo = st["xh"], st["xl"], st["whi"], st["wlo"]
    tp = slice(p * 256, (p + 1) * 256)

    def w_lhs(cols):
        def f(part, u):
            wt = wlo if part == "hl" else whi
            return wt[:, 2 * u : 2 * u + 2, cols]
        return f

    def x_rhs(part, u):
        xt = xl if part == "lh" else xh
        return xt[:, 2 * u : 2 * u + 2, tp]

    # za (n, 256): gate pre-activation, pure fp8 (hi only)
    za = st["ps_za"].tile([N, 256], F32, tag="za", name="za")
    for u in range(2):
        nc.tensor.matmul(za[:], st["wahi"][:, 2 * u : 2 * u + 2, :],
                         x_rhs("hh", u),
                         start=(u == 0), stop=(u == 1), perf_mode=DR)

    # KT | QT packed in one PSUM bank, hi/lo compensated
    kq = st["ps_kq"].tile([N, 512], F32, tag="kq", name="kq")
    _hilo_proj(nc, kq[:, 0:256], w_lhs(CK), x_rhs)
    _hilo_proj(nc, kq[:, 256:512], w_lhs(CQ), x_rhs)

    if mid is not None:
        mid()   # drain-tail interleave: stage_c of the previous pair

    # V per chunk (t, d), x64 scale retained (folded into the mask value).
    v_sb = []
    for h in range(2):
        tc_ = slice((2 * p + h) * C, (2 * p + h + 1) * C)

        def xc_lhs(part, u):
            xt = xl if part == "lh" else xh
            return xt[:, 2 * u : 2 * u + 2, tc_]

        def wv_rhs(part, u):
            wt = wlo if part == "hl" else whi
            return wt[:, 2 * u : 2 * u + 2, CV]

        vp = st["ps_v"].tile([C, D], F32, tag="v", name="v")
        _hilo_proj(nc, vp[:], xc_lhs, wv_rhs)
        vs = vout.tile([C, D], F32R, tag="vsb", name="vsb")
        # GPSIMD cannot touch PSUM -> V evacuations live on DVE
        if st["bv_full"] is None:
            nc.vector.tensor_copy(vs[:], vp[:])
        else:
            nc.vector.tensor_add(vs[:], vp[:], st["bv_full"][:])
        v_sb.append(vs)

    # gate chain: alpha = sigmoid(za/64 + ba) on ACT (per-partition bias).
    # High priority: this chain is the pair-to-pair latency path, so the
    # scheduler must slot it ahead of same-engine evacuation work.
    hp = st["tc"].high_priority(offset=150)
    hp.__enter__()
    alpha = work.tile([N, 256], F32, tag="alpha", name="alpha")
    nc.scalar.activation(alpha[:], za[:], AF.Sigmoid, bias=st["ba"], scale=1.0 / WS)
    cp = work.tile([N, 256], F32, tag="cp", name="cp")
    for h in range(2):
        hh = slice(h * C, (h + 1) * C)
        nc.vector.tensor_tensor_scan(
            cp[:, hh], alpha[:, hh], st["zeros"][:], 1.0, ALU.mult, ALU.add,
        )
    invp = work.tile([N, 256], F32, tag="invp", name="invp")
    nc.gpsimd.tensor_scalar_add(invp[:], cp[:], EPS)
    nc.vector.reciprocal_approx_fast(invp[:], invp[:])

    # k~ = (64k + 64bk) * invp ; q~ = (64q + 64bq) * cp.  Two-phase: ACT
    # evacuates the PSUM with the per-partition bias (Identity), Pool does
    # the SBUF-only gate multiply (GPSIMD may not touch PSUM).  The 64^2 in
    # k~*q~ and V's 64 are cancelled by UVAL = 64^-3.  The last pair uses
    # the direct (single-hop) DVE stt instead: at the drain, chain latency
    # matters more than DVE throughput.
    kt = gate.tile([N, 256], F32R, tag="kt", name="kt")
    qt = gate.tile([N, 256], F32R, tag="qt", name="qt")
    if p == NPAIR - 1:
        # drain: single-hop DVE stt minimizes the exposed chain latency
        nc.vector.scalar_tensor_tensor(kt[:], kq[:, 0:256], st["bk64"], invp[:],
                                       ALU.add, ALU.mult)
        nc.vector.scalar_tensor_tensor(qt[:], kq[:, 256:512], st["bq64"], cp[:],
                                       ALU.add, ALU.mult)
    else:
        kb = work.tile([N, 256], F32, tag="kb", name="kb")
        nc.scalar.activation(kb[:], kq[:, 0:256], AF.Identity, bias=st["bk64"])
        nc.gpsimd.tensor_mul(kt[:], kb[:], invp[:])
        qb = work.tile([N, 256], F32, tag="qb", name="qb")
        nc.scalar.activation(qb[:], kq[:, 256:512], AF.Identity, bias=st["bq64"])
        nc.gpsimd.tensor_mul(qt[:], qb[:], cp[:])
    hp.__exit__(None, None, None)

    return {"p": p, "kt": kt, "qt": qt, "v": v_sb}


def _emit_stage_c(nc, pst, st):
    """Intra-chunk attention + output for the pair produced by stage A."""
    p = pst["p"]
    last = p == NPAIR - 1
    ys = st["yout"].tile([C, 2, D], F16, tag="ysb", name="ysb")
    # AT_h = k~_h^T @ q~ over the FULL pair (free dim 256 keeps f32r at
    # 1 cyc/row); the mask-mult only reads the causal [s, t-in-chunk-h]
    # block.  atm h0 on DVE, h1 on Pool (parallel engines).
    at = st["ps_at"].tile([C, 2, 256], F32, tag="at", name="at")
    atms = []
    for h in range(2):
        hh = slice(h * C, (h + 1) * C)
        nc.tensor.matmul(at[:, h, :], pst["kt"][:, hh], pst["qt"][:],
                         start=True, stop=True)
        atm = st["work"].tile([C, C], F32R, tag="atm", name="atm")
        nc.vector.tensor_mul(atm[:], at[:, h, hh], st["U"][:])
        atms.append(atm)
    for h in range(2):
        yp = st["ps_y"].tile([C, D], F32, tag="y", name="y")
        nc.tensor.matmul(yp[:], atms[h][:], pst["v"][h][:], start=True, stop=True)
        if p >= NPAIR - 2 and h == 1:
            nc.vector.tensor_copy(ys[:, h, :], yp[:])  # drain: h1 on DVE
        else:
            with st["tc"].high_priority(offset=90):
            nc.scalar.copy(ys[:, h, :], yp[:])  # y evacs on ACT
        # per-chunk output DMA on alternating HWDGE queues: each chunk ships
        # as soon as its evacuation lands, and the final chunk's DMA chain
        # (gen+transfer+sem) is as short as possible.
        c = 2 * p + h
        # h1 is the later chunk: give it the SP queue (dge delay 650 vs the
        # ACT queue's 784) so the final DMA chain is shortest
        q = nc.scalar if h == 0 else nc.sync
        q.dma_start(st["y"][c * C : (c + 1) * C, :], ys[:, h, :])


_NC_CACHE = {}


def _get_nc(bv_zero=True):
    if bv_zero not in _NC_CACHE:
        _NC_CACHE[bv_zero] = build_nc(bv_zero)
    return _NC_CACHE[bv_zero]


def make_in_maps(x, Wv, bv, Wk, bk, Wq, bq, Wa, ba):
    import ml_dtypes

    e4, e5 = ml_dtypes.float8_e4m3, ml_dtypes.float8_e5m2
    x = np.asarray(x, np.float32)

    Wall = np.concatenate(
        [np.asarray(Wv, np.float32), np.asarray(Wk, np.float32),
         np.asarray(Wq, np.float32)], axis=0)
    WT64 = np.ascontiguousarray(Wall.T) * np.float32(WS)     # (I, 768)
    whi = WT64.astype(e4)
    wlo = (WT64 - whi.astype(np.float32)).astype(e5)
    wahi = (np.ascontiguousarray(np.asarray(Wa, np.float32).T)
            * np.float32(WS)).astype(e4)                     # (I, 128)

    bias = np.stack([
        np.asarray(ba, np.float32),
        np.asarray(bk, np.float32) * np.float32(WS),
        np.asarray(bq, np.float32) * np.float32(WS),
    ], axis=1)                                               # (N, 3)

    shared = {
        "wahi": np.ascontiguousarray(wahi),
        "whi": np.ascontiguousarray(whi),
        "wlo": np.ascontiguousarray(wlo),
        "bias": np.ascontiguousarray(bias),
        "bv64": (np.asarray(bv, np.float32) * np.float32(WS)).reshape(1, D),
    }
    in_maps = []
    for b in range(NCORES):
        xT = np.ascontiguousarray(x[:, b, :].T)              # (I, T)
        xh = xT.astype(e4)
        xlr = (xT - xh.astype(np.float32)).astype(e5)
        in_maps.append({"xh": np.ascontiguousarray(xh),
                        "xl": np.ascontiguousarray(xlr), **shared})
    return in_maps


def run(inputs, trace=False, **kw):
    bv_zero = not np.any(np.asarray(inputs["bv"]))
    nc = _get_nc(bv_zero)
    in_maps = make_in_maps(**inputs)
    res = run_bass_kernel_spmd(nc, in_maps, core_ids=list(range(NCORES)),
                               trace=trace, **kw)
    out = np.stack(
        [np.asarray(res.results[b]["y"], dtype=np.float32) for b in range(NCORES)],
        axis=1)
    return out, res


def kernel(x, Wv, bv, Wk, bk, Wq, bq, Wa, ba):
    out, _ = run(dict(x=x, Wv=Wv, bv=bv, Wk=Wk, bk=bk, Wq=Wq, bq=bq,
                      Wa=Wa, ba=ba))
    return out
